# revision 30
# baseline (speedup 1.0000x reference)
"""Multi-head attention on 8 Trainium2 NeuronCores.

Sharding: tensor-parallel over heads (2 heads/core), full batch on every
core; host sums the 8 partial outputs and adds b_o + b_v @ w_o.

Design (measured down from the 611us baseline to ~400us):
  A: xT[kt] [128, S] fp16 <- DMA of host-pretransposed x.
  B: Q^T/K^T = w^T xT + b (PSUM f32, bias fused in DVE copy -> fp16).
     V natural via lhsT = xT tile, rhs = w_v tile; DVE copy scales by
     V_SCALE into fp8e4 v8 [128, h, kt, 80] (ones col 64 = softmax
     denominator trick; 80-stride keeps the DoubleRow Ko step 16B).
  C: both heads per 512-wide query chunk, kt-paired:
     - two K=64 score MMs into disjoint PE row groups (rows 0-63 /
       64-127) with outputs in different PSUM banks of one [128,1024]
       st tile; alternating row groups lets each LDWEIGHTS pull ahead
       during the other head's MM, so scores run at the streaming floor.
     - one ACT exp per kt covers both heads, writing fp8e4 into a
       kt-pair tile; AV is a per-head fp8 DoubleRow MM (2 rows/cycle)
       accumulating [65, 512] in PSUM, lagging scores by 2 pairs so the
       chunk-boundary av readback never stalls the PE FIFO.
     - per chunk: av row 64 -> rline; 1/r via [1,512] -> DMA [16,32] ->
       DVE reciprocal -> DMA -> gpsimd partition_broadcast [128,512] ->
       DVE in-place attnT *= rb. The very last chunk uses a
       latency-optimized chain (reciprocal_approx_fast, no DMA hops).
     - stage-D units are queued per chunk and drained 1/slot, keeping a
       ~1-chunk lag so their normalization is always complete.
  D: po [128,512] = attnT_tile^T @ (w_o/V_SCALE) single K=128 MM; DVE
     cast -> fp16 -> DMA out.

fp8 path (pexp + V in e4m3, validated vs the fp32 reference):
rel_err 1.44e-2 deterministic against the 2e-2 gate; the fp16-everywhere
variant measures 3.8e-4 but runs ~15% slower (AV at 1 row/cycle).
"""

import numpy as np

import concourse.bacc as bacc
import concourse.mybir as mybir
from concourse.tile import TileContext
from concourse import bass_utils

dt = mybir.dt
F32 = dt.float32
F16 = dt.float16
F8 = dt.float8e4
AF = mybir.ActivationFunctionType
ALU = mybir.AluOpType
PM = mybir.MatmulPerfMode

USE_FP8_AV = True          # pexp & V in fp8e4, AV via DoubleRow (2x PE rate)
V_SCALE = 16.0             # V is scaled by this before fp8; w_o divided on host

B, S, D = 4, 2048, 1024
H, DH = 16, 64
NCORES = 8
HPC = H // NCORES          # heads per core = 2
DHC = HPC * DH             # 128 projection cols per core

D_SKID = 2                 # C-slots to skip before draining stage D

_CACHE = {}


def build_nc(b=B, s=S):
    d = D
    n_tt = s // 128            # token tiles per batch
    n_kt = d // 128            # contraction tiles for projections
    qw = 1024                  # B-stage projection chunk width
    assert s % 1024 == 0 and d == 1024
    assert USE_FP8_AV, "stage C is fp8-DoubleRow only"

    nc = bacc.Bacc("TRN2", target_bir_lowering=False, debug=False)

    x_d = nc.dram_tensor("x", [b, d, s], F16, kind="ExternalInput")
    wq_d = nc.dram_tensor("wq", [d, DHC], F16, kind="ExternalInput")
    wk_d = nc.dram_tensor("wk", [d, DHC], F16, kind="ExternalInput")
    wv_d = nc.dram_tensor("wv", [d, DHC], F16, kind="ExternalInput")
    bq_d = nc.dram_tensor("bq", [DHC, 1], F32, kind="ExternalInput")
    bk_d = nc.dram_tensor("bk", [DHC, 1], F32, kind="ExternalInput")
    wo_d = nc.dram_tensor("wo", [DHC, d], F16, kind="ExternalInput")
    out_d = nc.dram_tensor("out", [b, s, d], F16, kind="ExternalOutput")

    with TileContext(nc) as tc:
        with (
            tc.tile_pool(name="const", bufs=1) as cpool,
            tc.tile_pool(name="wpool", bufs=3 * n_kt) as wpool,
            tc.tile_pool(name="xt", bufs=2 * n_kt) as xt_pool,
            tc.tile_pool(name="qk", bufs=2) as qk_pool,
            tc.tile_pool(name="vt", bufs=3) as vt_pool,
            tc.tile_pool(name="at", bufs=2) as at_pool,
            tc.tile_pool(name="pexp", bufs=3) as pexp_pool,
            tc.tile_pool(name="rline", bufs=2) as rline_pool,
            tc.tile_pool(name="rb", bufs=2) as rb_pool,
            tc.tile_pool(name="small", bufs=4) as small,
            tc.tile_pool(name="osb", bufs=6) as osb_pool,
            tc.tile_pool(name="ps", bufs=1, space="PSUM") as pp,
        ):
            # ---- stage-A DMA for batch 0 first: the first projection
            # matmul needs only xT[0] + wq[0], so interleave x and weight
            # loads instead of front-loading all weights.
            xT_all = {}
            for kt in range(n_kt):
                xt = xt_pool.tile([128, s], F16, tag="xt", name=f"xT0_{kt}")
                nc.sync.dma_start(out=xt[:, :],
                                  in_=x_d[0, kt * 128:(kt + 1) * 128, :])
                xT_all[(0, kt)] = xt

            w16 = {}
            for kt in range(n_kt):
                for name, dram in (("q", wq_d), ("k", wk_d), ("v", wv_d)):
                    wt = wpool.tile([128, DHC], F16, tag="w",
                                    name=f"w_{name}{kt}")
                    nc.sync.dma_start(
                        out=wt[:, :], in_=dram[kt * 128:(kt + 1) * 128, :]
                    )
                    w16[(name, kt)] = wt

            ones_col = cpool.tile([128, 32], F16, tag="ones_col")
            nc.vector.memset(ones_col[:, :], 1.0)
            bq = cpool.tile([DHC, 1], F32, tag="bq")
            bk = cpool.tile([DHC, 1], F32, tag="bk")
            nc.sync.dma_start(out=bq[:, :], in_=bq_d[:, :])
            nc.sync.dma_start(out=bk[:, :], in_=bk_d[:, :])
            wo = cpool.tile([DHC, d], F16, tag="wo")
            nc.sync.dma_start(out=wo[:, :], in_=wo_d[:, :])

            # stage-D state carried across the batch loop
            d_queue = []    # pending stage-D units for prev batch
            slot = [0]      # C-slot counter within the current batch

            def emit_d_unit(n=None):
                """Emit queued (bi, tt, half) output units. Draining ~1 per
                C-slot keeps a one-chunk lag behind the normalization chain;
                drain 2 only when backlogged so the final flush stays short."""
                slot[0] += 1
                if slot[0] <= D_SKID:
                    return
                if n is None:
                    n = 2 if len(d_queue) > 12 else 1
                for _ in range(min(n, len(d_queue))):
                    attnT_p, bi_out, tt, half = d_queue.pop(0)
                    cs = slice(half * 512, (half + 1) * 512)
                    po = pp.tile([128, 512], F32, tag="po", bufs=2, name="po")
                    nc.tensor.matmul(
                        po[:, :], attnT_p[:, tt * 128:(tt + 1) * 128],
                        wo[:, cs], start=True, stop=True,
                    )
                    osb = osb_pool.tile([128, 512], F16, tag="osb", name="osb")
                    nc.vector.tensor_copy(osb[:, :], po[:, :])
                    nc.sync.dma_start(
                        out=out_d[bi_out, tt * 128:(tt + 1) * 128, cs],
                        in_=osb[:, :],
                    )

            for bi in range(b):
                # ---- stage A: xT tiles (host pre-transposed x) ----
                if bi > 0:
                    for kt in range(n_kt):
                        xt = xt_pool.tile([128, s], F16, tag="xt",
                                          name=f"xT{bi}_{kt}")
                        nc.sync.dma_start(
                            out=xt[:, :],
                            in_=x_d[bi, kt * 128:(kt + 1) * 128, :],
                        )
                        xT_all[(bi, kt)] = xt
                xT = [xT_all[(bi, kt)] for kt in range(n_kt)]

                # ---- stage B: Q^T, K^T projections ----
                qT = qk_pool.tile([DHC, s], F16, tag="qT")
                kT = qk_pool.tile([DHC, s], F16, tag="kT")
                for name, dst, bias in (("q", qT, bq), ("k", kT, bk)):
                    for c in range(s // qw):
                        ppr = pp.tile([128, qw], F32, tag="st", bufs=2,
                                      name="ppr")
                        for kt in range(n_kt):
                            for j in range(qw // 512):
                                nc.tensor.matmul(
                                    ppr[:, j * 512:(j + 1) * 512],
                                    w16[(name, kt)][:, :],
                                    xT[kt][:, c * qw + j * 512:
                                            c * qw + (j + 1) * 512],
                                    start=(kt == 0),
                                    stop=(kt == n_kt - 1),
                                )
                        nc.vector.tensor_scalar_add(
                            dst[:, c * qw:(c + 1) * qw], ppr[:, :], bias[:, 0:1]
                        )
                # V natural. fp8: [128, h, kt, 80] with V*16 in cols 0-63 and
                # the ones column (softmax denominator trick) at col 64; the
                # 80-stride keeps the DoubleRow Ko step 16B-aligned.
                # fp16: interleaved-head layout [V_A |1| V_B |1] per 130 cols.
                if USE_FP8_AV:
                    v8 = vt_pool.tile([128, 2 * n_tt * 80], F8, tag="vt")
                    v8r = v8.rearrange("p (h t c) -> p h t c", h=2, c=80)
                    nc.vector.memset(v8r[:, :, :, 64], 1.0)
                else:
                    vt = vt_pool.tile([128, n_tt * 130], F16, tag="vt")
                    ones_dst = vt.rearrange("p (t two sv) -> p t two sv",
                                            two=2, sv=65)[:, :, :, 64]
                    nc.vector.tensor_copy(
                        ones_dst, ones_col[:, 0:2 * n_tt]
                        .rearrange("p (t two) -> p t two", two=2))
                for tt in range(n_tt):
                    pv = pp.tile([128, 128], F32, tag="po", bufs=2, name="pv")
                    for kt in range(n_kt):
                        nc.tensor.matmul(
                            pv[:, :],
                            xT[kt][:, tt * 128:(tt + 1) * 128],
                            w16[("v", kt)][:, :],
                            start=(kt == 0),
                            stop=(kt == n_kt - 1),
                        )
                    if USE_FP8_AV:
                        for h in range(HPC):
                            nc.vector.tensor_scalar_mul(
                                v8r[:, h, tt, 0:64],
                                pv[:, h * 64:(h + 1) * 64], V_SCALE)
                    else:
                        vdst = vt.rearrange("p (t two sv) -> p t two sv",
                                            two=2, sv=65)[:, tt, :, 0:64]
                        nc.vector.tensor_copy(
                            vdst, pv.rearrange("p (two sv) -> p two sv", two=2)
                        )

                # ---- stage C: attention (+ interleaved stage D of bi-1) ----
                slot[0] = 0
                attnT = at_pool.tile([DHC, s], F16, tag="attnT")
                rlines = [rline_pool.tile([1, s], F32, tag="rline",
                                          name=f"rline{h}")
                          for h in range(HPC)]
                rrls = [small.tile([1, s], F16, tag="rrl", bufs=2,
                                   name=f"rrl{h}") for h in range(HPC)]
                rbs = [rb_pool.tile([128, s], F16, tag="rb",
                                    name=f"rb{h}") for h in range(HPC)]
                # Both heads together per 512-wide query chunk: the two
                # K=64 score MMs go to disjoint PE row groups (rows 0-63 /
                # 64-127 via base partitions) with outputs in different PSUM
                # banks, so they CO-EXECUTE (~2x score throughput). One exp
                # covers both heads' halves; AV is a per-head fp8 DoubleRow
                # MM over kt-pairs.
                for c in range(s // 512):
                    cq = slice(c * 512, (c + 1) * 512)
                    av = [pp.tile([65, 512], F32, tag=f"av{h}",
                                  name=f"av{h}") for h in range(HPC)]
                    pexps = {}
                    # AV lags the scores by 2 kt-pairs so the first AV of a
                    # chunk never stalls the PE FIFO on the previous chunk's
                    # av-readback (attnT/rline copies).
                    LAG = 2
                    for kp in range(n_tt // 2 + LAG):
                        if kp < n_tt // 2:
                            sts = {}
                            for kt in (2 * kp, 2 * kp + 1):
                                st = pp.tile([128, 1024], F32, tag="st",
                                             bufs=2, name="st")
                                for h in range(HPC):
                                    hs = slice(h * 64, (h + 1) * 64)
                                    nc.tensor.matmul(
                                        st[:, h * 512:(h + 1) * 512],
                                        kT[hs, kt * 128:(kt + 1) * 128],
                                        qT[hs, cq],
                                        start=True, stop=True,
                                    )
                                sts[kt] = st
                            emit_d_unit()
                            px8 = pexp_pool.tile(
                                [128, 2 * 1024], F8, tag="pexp",
                                bufs=4, name="pexp")
                            for kt in (2 * kp, 2 * kp + 1):
                                nc.scalar.activation(
                                    px8[:, (kt % 2) * 1024:
                                        (kt % 2 + 1) * 1024],
                                    sts[kt][:, :], AF.Exp, scale=0.125
                                )
                            pexps[kp] = px8.rearrange(
                                "p (two h q) -> p two h q", two=2, h=2)
                        if kp >= LAG:
                            px8r = pexps.pop(kp - LAG)
                            for h in range(HPC):
                                nc.tensor.matmul(
                                    av[h][:, :],
                                    v8r[:, h, 2 * (kp - LAG):
                                        2 * (kp - LAG) + 2, 0:65],
                                    px8r[:, :, h, :],
                                    start=(kp == LAG),
                                    stop=(kp == n_tt // 2 + LAG - 1),
                                    perf_mode=PM.DoubleRow,
                                )
                    # per-chunk softmax renorm chain; D units of this chunk
                    # become available a few us later, so the queue can
                    # drain inside the SAME batch (fills batch 0, trims the
                    # final-batch flush stall):
                    # [1,512] --dma--> [4,128] --recip--> fp16
                    # --dma--> [1,512] --gpsimd bcast--> [128,512] --mul
                    fast_tail = (c == s // 512 - 1)
                    if fast_tail:
                        # latency-optimized: skip the DMA reshape hops and
                        # front-load both heads' reciprocals so the gpsimd
                        # broadcasts overlap the attnT casts — the final D
                        # units start ~7us sooner
                        rrl32s, rb32s = [], []
                        for h in range(HPC):
                            nc.vector.tensor_copy(rlines[h][0:1, cq],
                                                  av[h][64:65, :])
                        for h in range(HPC):
                            rrl32 = small.tile([1, 512], F32, tag="rrl32")
                            nc.vector.reciprocal_approx_fast(
                                rrl32[0:1, :], rlines[h][0:1, cq])
                            rrl32s.append(rrl32)
                            rb32 = small.tile([128, 512], F32, tag="rb32")
                            nc.gpsimd.partition_broadcast(
                                rb32[:, :], rrl32[0:1, :], channels=128)
                            rb32s.append(rb32)
                        for h in range(HPC):
                            hs = slice(h * 64, (h + 1) * 64)
                            nc.vector.tensor_copy(attnT[hs, cq],
                                                  av[h][0:64, :])
                            nc.vector.tensor_mul(attnT[hs, cq],
                                                 attnT[hs, cq],
                                                 rb32s[h][hs, :])
                        for tt in range(4 * c, 4 * c + 4):
                            for half in range(2):
                                d_queue.append((attnT, bi, tt, half))
                        continue
                    for h in range(HPC):
                        hs = slice(h * 64, (h + 1) * 64)
                        nc.vector.tensor_copy(rlines[h][0:1, cq],
                                              av[h][64:65, :])
                        nc.vector.tensor_copy(attnT[hs, cq], av[h][0:64, :])
                        r16 = small.tile([16, 32], F32, tag="r16")
                        nc.sync.dma_start(out=r16[:, :],
                                          in_=rlines[h][0:1, cq])
                        rr16 = small.tile([16, 32], F16, tag="rr16")
                        with nc.allow_low_precision(reason="1/r fp16 ok"):
                            nc.vector.reciprocal(rr16[:, :], r16[:, :])
                        nc.sync.dma_start(out=rrls[h][0:1, cq],
                                          in_=rr16[:, :])
                        nc.gpsimd.partition_broadcast(
                            rbs[h][:, cq], rrls[h][0:1, cq], channels=128)
                        nc.vector.tensor_mul(attnT[hs, cq], attnT[hs, cq],
                                             rbs[h][hs, cq])
                    for tt in range(4 * c, 4 * c + 4):
                        for half in range(2):
                            d_queue.append((attnT, bi, tt, half))

            # flush the remaining stage-D units
            slot[0] = D_SKID + 1
            while d_queue:
                emit_d_unit()

    nc.compile()
    return nc


def _get_nc(b, s):
    key = (b, s)
    if key not in _CACHE:
        _CACHE[key] = build_nc(b, s)
    return _CACHE[key]


def make_in_maps(x, w_q, b_q, w_k, b_k, w_v, w_o):
    x16 = np.ascontiguousarray(
        np.asarray(x, dtype=np.float16).transpose(0, 2, 1))
    wq16 = np.asarray(w_q, dtype=np.float16)
    wk16 = np.asarray(w_k, dtype=np.float16)
    wv16 = np.asarray(w_v, dtype=np.float16)
    wo_scale = 1.0 / V_SCALE if USE_FP8_AV else 1.0
    wo16 = np.asarray(w_o * wo_scale, dtype=np.float16)
    in_maps = []
    for i in range(NCORES):
        cs = slice(i * DHC, (i + 1) * DHC)
        in_maps.append({
            "x": x16,
            "wq": np.ascontiguousarray(wq16[:, cs]),
            "wk": np.ascontiguousarray(wk16[:, cs]),
            "wv": np.ascontiguousarray(wv16[:, cs]),
            "bq": np.ascontiguousarray(b_q[cs, None], dtype=np.float32),
            "bk": np.ascontiguousarray(b_k[cs, None], dtype=np.float32),
            "wo": np.ascontiguousarray(wo16[cs, :]),
        })
    return in_maps


def kernel(x, w_q, b_q, w_k, b_k, w_v, b_v, w_o, b_o, _trace=False):
    x = np.asarray(x, dtype=np.float32)
    nc = _get_nc(x.shape[0], x.shape[1])
    in_maps = make_in_maps(x, w_q, b_q, w_k, b_k, w_v, w_o)
    kw = {}
    if _trace:
        import tempfile
        kw = dict(trace=True, trace_cores=list(range(NCORES)),
                  tmpdir=tempfile.mkdtemp(prefix="mha_trace_"))
    res = bass_utils.run_bass_kernel_spmd(
        nc, in_maps, core_ids=list(range(NCORES)), **kw
    )
    out = np.zeros(x.shape, dtype=np.float32)
    for i in range(NCORES):
        out += np.asarray(res.results[i]["out"], dtype=np.float32)
    out += np.asarray(b_o, dtype=np.float32)[None, None, :]
    out += (np.asarray(b_v, dtype=np.float32)
            @ np.asarray(w_o, dtype=np.float32))[None, None, :]
    if _trace:
        return out, res
    return out
